# revision 11
# baseline (speedup 1.0000x reference)
import sys
import os
import time
import numpy as np
from contextlib import ExitStack

sys.path.insert(0, '/opt/trn_rl_repo')

import concourse.bass as bass
from concourse import bacc
import concourse.mybir as mybir
import concourse.tile as tile
from concourse.bass import ds
from concourse.bass_utils import run_bass_kernel_spmd

V, E, H, S, T = 50000, 1024, 1024, 4096, 5
Hh = H // 2            # 512
G = 4 * Hh             # 2048 gates per direction
GB = 2 * G             # 4096 both directions stacked
NCORES = 8
SC = S // NCORES       # 512 timesteps per core for the projection GEMM
JH = Hh // 128         # 4 hidden chunks
GM = G // 128          # 16 gate chunks
START, STOP = 3, 4
NEG = -10000.0
F32 = mybir.dt.float32
AF = mybir.ActivationFunctionType
EXEC_NS = {}


def _build_proj():
    # Per core: xpT_chunk[GB, SC] = w_both @ x_chunk.T + b_both
    # inputs: wT = w_both.T [E, GB], xT = x_chunk.T [E, SC], bias [GB]
    nc = bacc.Bacc()
    wT = nc.declare_dram_parameter("wT", [E, GB], F32, isOutput=False)
    xT = nc.declare_dram_parameter("xT", [E, SC], F32, isOutput=False)
    bia = nc.declare_dram_parameter("bias", [GB], F32, isOutput=False)
    out = nc.declare_dram_parameter("xpT", [GB, SC], F32, isOutput=True)
    KT = E // 128      # 8
    MT = GB // 128     # 32
    with tile.TileContext(nc) as tc, ExitStack() as ctx:
        const = ctx.enter_context(tc.tile_pool(name="const", bufs=1))
        opool = ctx.enter_context(tc.tile_pool(name="o", bufs=3))
        ppool = ctx.enter_context(tc.tile_pool(name="p", bufs=2, space="PSUM"))
        b_sb = const.tile([128, MT], F32)
        nc.sync.dma_start(b_sb[:, :], bia.rearrange("(m p) -> p m", p=128))
        x_sb = const.tile([128, KT, SC], F32)
        for k in range(KT):
            nc.sync.dma_start(x_sb[:, k, :], xT[k * 128:(k + 1) * 128, :])
        w_all = const.tile([128, KT, GB], F32)   # 128 KiB/partition
        for k in range(KT):
            nc.sync.dma_start(w_all[:, k, :], wT[k * 128:(k + 1) * 128, :])
        for m in range(MT):
            ps = ppool.tile([128, SC], F32)
            for k in range(KT):
                nc.tensor.matmul(ps[:, :], w_all[:, k, m * 128:(m + 1) * 128],
                                 x_sb[:, k, :],
                                 start=(k == 0), stop=(k == KT - 1))
            o_sb = opool.tile([128, SC], F32, tag="o")
            nc.scalar.activation(o_sb[:, :], ps[:, :], AF.Identity, bias=b_sb[:, m:m + 1])
            nc.sync.dma_start(out[m * 128:(m + 1) * 128, :], o_sb[:, :])
    nc.compile()
    return nc


def _build_rec():
    # One LSTM direction per core (SPMD: same graph, different data).
    # xpT [G, S]  : input projections (bias included), time-reversed for bwd core
    # whhT [Hh, G]: w_hh.T
    # hc0 [128, 2*JH]: h0 | c0 arranged (p, j) -> hidden j*128+p
    # out hsT [128, 4*S]: step t cols [4t,4t+4), col 4t+j partition p = h_t[j*128+p]
    nc = bacc.Bacc()
    xpT = nc.declare_dram_parameter("xpT", [G, S], F32, isOutput=False)
    whhT = nc.declare_dram_parameter("whhT", [Hh, G], F32, isOutput=False)
    hc0 = nc.declare_dram_parameter("hc0", [128, 2 * JH], F32, isOutput=False)
    out = nc.declare_dram_parameter("hsT", [128, 4 * S], F32, isOutput=True)
    with tile.TileContext(nc) as tc, ExitStack() as ctx:
        const = ctx.enter_context(tc.tile_pool(name="const", bufs=1))
        xpool = ctx.enter_context(tc.tile_pool(name="xp", bufs=4))
        work = ctx.enter_context(tc.tile_pool(name="work", bufs=2))
        ppool = ctx.enter_context(tc.tile_pool(name="ps", bufs=2, space="PSUM"))
        W_sb = const.tile([128, JH, G], F32)   # 32 KiB/partition
        for j in range(JH):
            nc.sync.dma_start(W_sb[:, j, :], whhT[j * 128:(j + 1) * 128, :])
        h_sb = const.tile([128, JH], F32)
        c_sb = const.tile([128, JH], F32)
        nc.sync.dma_start(h_sb[:, :], hc0[:, 0:JH])
        nc.sync.dma_start(c_sb[:, :], hc0[:, JH:2 * JH])
        xp_r = xpT.rearrange("(m p) s -> p m s", p=128)   # [128, GM, S]
        with tc.For_i(0, S, 1) as i:
            xp_sb = xpool.tile([128, GM], F32, tag="xp")
            nc.sync.dma_start(xp_sb[:, :], xp_r[:, :, ds(i, 1)])
            g_ps = ppool.tile([128, GM], F32, tag="g")
            for m in range(GM):
                for j in range(JH):
                    nc.tensor.matmul(g_ps[:, m:m + 1],
                                     W_sb[:, j, m * 128:(m + 1) * 128],
                                     h_sb[:, j:j + 1],
                                     start=(j == 0), stop=(j == JH - 1))
            a_sb = work.tile([128, GM], F32, tag="a")
            nc.vector.tensor_add(a_sb[:, :], g_ps[:, :], xp_sb[:, :])
            act_sb = work.tile([128, GM], F32, tag="act")
            nc.scalar.activation(act_sb[:, 0:2 * JH], a_sb[:, 0:2 * JH], AF.Sigmoid)
            nc.scalar.activation(act_sb[:, 2 * JH:3 * JH], a_sb[:, 2 * JH:3 * JH], AF.Tanh)
            nc.scalar.activation(act_sb[:, 3 * JH:4 * JH], a_sb[:, 3 * JH:4 * JH], AF.Sigmoid)
            t1 = work.tile([128, JH], F32, tag="t1")
            nc.vector.tensor_mul(t1[:, :], act_sb[:, JH:2 * JH], c_sb[:, :])       # f*c
            t2 = work.tile([128, JH], F32, tag="t2")
            nc.vector.tensor_mul(t2[:, :], act_sb[:, 0:JH], act_sb[:, 2 * JH:3 * JH])  # i*g
            nc.vector.tensor_add(c_sb[:, :], t1[:, :], t2[:, :])
            t3 = work.tile([128, JH], F32, tag="t3")
            nc.scalar.activation(t3[:, :], c_sb[:, :], AF.Tanh)
            nc.vector.tensor_mul(h_sb[:, :], act_sb[:, 3 * JH:4 * JH], t3[:, :])
            nc.sync.dma_start(out[:, ds(i * 4, 4)], h_sb[:, :])
    nc.compile()
    return nc


def _arr_hidden(v):
    # [Hh] -> [128, JH] with (p, j) = hidden j*128+p
    return np.ascontiguousarray(v.reshape(JH, 128).T)


def _decode_hsT(o):
    # [128, 4*S] -> [S, Hh]
    return o.reshape(128, S, JH).transpose(1, 2, 0).reshape(S, Hh)


def kernel(embed_table, w_ih_f, w_hh_f, b_f, w_ih_b, w_hh_b, b_b,
           W_out, b_out, transitions, h0, c0, sentence):
    embed_table = np.asarray(embed_table, np.float32)
    sentence = np.asarray(sentence).astype(np.int64)
    x = embed_table[sentence]                      # [S, E] host gather
    xT = np.ascontiguousarray(x.T)                 # [E, S]

    w_both = np.concatenate([np.asarray(w_ih_f, np.float32),
                             np.asarray(w_ih_b, np.float32)], axis=0)  # [GB, E]
    wT = np.ascontiguousarray(w_both.T)            # [E, GB]
    b_both = np.concatenate([np.asarray(b_f, np.float32),
                             np.asarray(b_b, np.float32)])             # [GB]

    # ---- NEFF A: input projections, time-sharded over 8 cores ----
    nc_a = _build_proj()
    in_maps = [{"wT": wT, "bias": b_both,
                "xT": np.ascontiguousarray(xT[:, c * SC:(c + 1) * SC])}
               for c in range(NCORES)]
    res_a = run_bass_kernel_spmd(nc_a, in_maps, core_ids=list(range(NCORES)))
    EXEC_NS['proj'] = res_a.exec_time_ns
    if os.environ.get('BASS_BENCH'):
        t0 = time.time()
        run_bass_kernel_spmd(nc_a, in_maps, core_ids=list(range(NCORES)))
        EXEC_NS['proj_wall2'] = int((time.time() - t0) * 1e9)
    xpT_all = np.concatenate([res_a.results[c]["xpT"] for c in range(NCORES)], axis=1)  # [GB, S]
    xpT_f = np.ascontiguousarray(xpT_all[:G])
    xpT_b = np.ascontiguousarray(xpT_all[G:, ::-1])   # reversed time for bwd

    # ---- NEFF B: the two recurrences on cores 0 (fwd) and 1 (bwd) ----
    nc_b = _build_rec()
    h0 = np.asarray(h0, np.float32)
    c0 = np.asarray(c0, np.float32)
    maps_b = []
    for (xpT_d, whh, hrow, crow) in ((xpT_f, w_hh_f, h0[0], c0[0]),
                                     (xpT_b, w_hh_b, h0[1], c0[1])):
        maps_b.append({
            "xpT": xpT_d,
            "whhT": np.ascontiguousarray(np.asarray(whh, np.float32).T),
            "hc0": np.ascontiguousarray(
                np.concatenate([_arr_hidden(hrow), _arr_hidden(crow)], axis=1)),
        })
    res_b = run_bass_kernel_spmd(nc_b, maps_b, core_ids=[0, 1])
    EXEC_NS['rec'] = res_b.exec_time_ns
    if os.environ.get('BASS_BENCH'):
        t0 = time.time()
        run_bass_kernel_spmd(nc_b, maps_b, core_ids=[0, 1])
        EXEC_NS['rec_wall2'] = int((time.time() - t0) * 1e9)
    hf = _decode_hsT(res_b.results[0]["hsT"])          # [S, Hh]
    hb = _decode_hsT(res_b.results[1]["hsT"])[::-1]    # un-reverse

    # ---- host epilogue: output projection + Viterbi + backtrace ----
    feats = np.concatenate([hf, hb], axis=1) @ np.asarray(W_out, np.float32).T \
        + np.asarray(b_out, np.float32)                # [S, T]
    trans = np.asarray(transitions, np.float32)
    fv = np.full(T, NEG, np.float32)
    fv[START] = 0.0
    bptrs = np.zeros((S, T), np.int32)
    for t in range(S):
        scores = fv[None, :] + trans                   # [next, prev]
        bptrs[t] = scores.argmax(1)
        fv = scores.max(1).astype(np.float32) + feats[t]
    terminal = fv + trans[STOP]
    best = int(terminal.argmax())
    score = np.float32(terminal[best])
    path = np.zeros(S, np.int32)
    b = best
    for t in range(S - 1, -1, -1):
        path[t] = b
        b = bptrs[t, b]
    return np.array(score, np.float32), path


# revision 12
# speedup vs baseline: 1.0352x; 1.0352x over previous
import sys
import os
import time
import numpy as np
from contextlib import ExitStack

sys.path.insert(0, '/opt/trn_rl_repo')

import concourse.bass as bass
from concourse import bacc
import concourse.mybir as mybir
import concourse.tile as tile
from concourse.bass import ds
from concourse.bass_utils import run_bass_kernel_spmd

V, E, H, S, T = 50000, 1024, 1024, 4096, 5
Hh = H // 2            # 512
G = 4 * Hh             # 2048 gates per direction
GB = 2 * G             # 4096 both directions stacked
NCORES = 8
SC = S // NCORES       # 512 timesteps per core for the projection GEMM
JH = Hh // 128         # 4 hidden chunks
GM = G // 128          # 16 gate chunks
START, STOP = 3, 4
NEG = -10000.0
F32 = mybir.dt.float32
AF = mybir.ActivationFunctionType
EXEC_NS = {}


def _build_proj():
    # Per core: xpT_chunk[GB, SC] = w_both @ x_chunk.T + b_both
    # inputs: wT = w_both.T [E, GB], xT = x_chunk.T [E, SC], bias [GB]
    nc = bacc.Bacc()
    wT = nc.declare_dram_parameter("wT", [E, GB], F32, isOutput=False)
    xT = nc.declare_dram_parameter("xT", [E, SC], F32, isOutput=False)
    bia = nc.declare_dram_parameter("bias", [GB], F32, isOutput=False)
    out = nc.declare_dram_parameter("xpT", [GB, SC], F32, isOutput=True)
    KT = E // 128      # 8
    MT = GB // 128     # 32
    with tile.TileContext(nc) as tc, ExitStack() as ctx:
        const = ctx.enter_context(tc.tile_pool(name="const", bufs=1))
        opool = ctx.enter_context(tc.tile_pool(name="o", bufs=3))
        ppool = ctx.enter_context(tc.tile_pool(name="p", bufs=2, space="PSUM"))
        b_sb = const.tile([128, MT], F32)
        nc.sync.dma_start(b_sb[:, :], bia.rearrange("(m p) -> p m", p=128))
        x_sb = const.tile([128, KT, SC], F32)
        for k in range(KT):
            nc.sync.dma_start(x_sb[:, k, :], xT[k * 128:(k + 1) * 128, :])
        w_all = const.tile([128, KT, GB], F32)   # 128 KiB/partition
        for k in range(KT):
            nc.sync.dma_start(w_all[:, k, :], wT[k * 128:(k + 1) * 128, :])
        for m in range(MT):
            ps = ppool.tile([128, SC], F32)
            for k in range(KT):
                nc.tensor.matmul(ps[:, :], w_all[:, k, m * 128:(m + 1) * 128],
                                 x_sb[:, k, :],
                                 start=(k == 0), stop=(k == KT - 1))
            o_sb = opool.tile([128, SC], F32, tag="o")
            nc.scalar.activation(o_sb[:, :], ps[:, :], AF.Identity, bias=b_sb[:, m:m + 1])
            nc.sync.dma_start(out[m * 128:(m + 1) * 128, :], o_sb[:, :])
    nc.compile()
    return nc


def _build_rec():
    # One LSTM direction per core (SPMD: same graph, different data).
    # xpT [G, S]  : input projections (bias included), time-reversed for bwd core
    # whhT [Hh, G]: w_hh.T
    # hc0 [128, 2*JH]: h0 | c0 arranged (p, j) -> hidden j*128+p
    # out hsT [128, 4*S]: step t cols [4t,4t+4), col 4t+j partition p = h_t[j*128+p]
    nc = bacc.Bacc()
    xpT = nc.declare_dram_parameter("xpT", [G, S], F32, isOutput=False)
    whhT = nc.declare_dram_parameter("whhT", [Hh, G], F32, isOutput=False)
    hc0 = nc.declare_dram_parameter("hc0", [128, 2 * JH], F32, isOutput=False)
    out = nc.declare_dram_parameter("hsT", [128, 4 * S], F32, isOutput=True)
    with tile.TileContext(nc) as tc, ExitStack() as ctx:
        const = ctx.enter_context(tc.tile_pool(name="const", bufs=1))
        xpool = ctx.enter_context(tc.tile_pool(name="xp", bufs=4))
        work = ctx.enter_context(tc.tile_pool(name="work", bufs=2))
        ppool = ctx.enter_context(tc.tile_pool(name="ps", bufs=2, space="PSUM"))
        W_sb = const.tile([128, JH, G], F32)   # 32 KiB/partition
        for j in range(JH):
            nc.sync.dma_start(W_sb[:, j, :], whhT[j * 128:(j + 1) * 128, :])
        h_sb = const.tile([128, JH], F32)
        c_sb = const.tile([128, JH], F32)
        nc.sync.dma_start(h_sb[:, :], hc0[:, 0:JH])
        nc.sync.dma_start(c_sb[:, :], hc0[:, JH:2 * JH])
        xp_r = xpT.rearrange("(m p) s -> p m s", p=128)   # [128, GM, S]
        with tc.For_i(0, S, 1, staggered_reset=True) as i:
            xp_sb = xpool.tile([128, GM], F32, tag="xp")
            nc.sync.dma_start(xp_sb[:, :], xp_r[:, :, ds(i, 1)])
            g_ps = ppool.tile([128, GM], F32, tag="g")
            for m in range(GM):
                for j in range(JH):
                    nc.tensor.matmul(g_ps[:, m:m + 1],
                                     W_sb[:, j, m * 128:(m + 1) * 128],
                                     h_sb[:, j:j + 1],
                                     start=(j == 0), stop=(j == JH - 1))
            a_sb = work.tile([128, GM], F32, tag="a")
            nc.vector.tensor_add(a_sb[:, :], g_ps[:, :], xp_sb[:, :])
            act_sb = work.tile([128, GM], F32, tag="act")
            nc.scalar.activation(act_sb[:, 0:2 * JH], a_sb[:, 0:2 * JH], AF.Sigmoid)
            nc.scalar.activation(act_sb[:, 2 * JH:3 * JH], a_sb[:, 2 * JH:3 * JH], AF.Tanh)
            nc.scalar.activation(act_sb[:, 3 * JH:4 * JH], a_sb[:, 3 * JH:4 * JH], AF.Sigmoid)
            t1 = work.tile([128, JH], F32, tag="t1")
            nc.vector.tensor_mul(t1[:, :], act_sb[:, JH:2 * JH], c_sb[:, :])       # f*c
            t2 = work.tile([128, JH], F32, tag="t2")
            nc.vector.tensor_mul(t2[:, :], act_sb[:, 0:JH], act_sb[:, 2 * JH:3 * JH])  # i*g
            nc.vector.tensor_add(c_sb[:, :], t1[:, :], t2[:, :])
            t3 = work.tile([128, JH], F32, tag="t3")
            nc.scalar.activation(t3[:, :], c_sb[:, :], AF.Tanh)
            nc.vector.tensor_mul(h_sb[:, :], act_sb[:, 3 * JH:4 * JH], t3[:, :])
            nc.sync.dma_start(out[:, ds(i * 4, 4)], h_sb[:, :])
    nc.compile()
    return nc


def _arr_hidden(v):
    # [Hh] -> [128, JH] with (p, j) = hidden j*128+p
    return np.ascontiguousarray(v.reshape(JH, 128).T)


def _decode_hsT(o):
    # [128, 4*S] -> [S, Hh]
    return o.reshape(128, S, JH).transpose(1, 2, 0).reshape(S, Hh)


def kernel(embed_table, w_ih_f, w_hh_f, b_f, w_ih_b, w_hh_b, b_b,
           W_out, b_out, transitions, h0, c0, sentence):
    embed_table = np.asarray(embed_table, np.float32)
    sentence = np.asarray(sentence).astype(np.int64)
    x = embed_table[sentence]                      # [S, E] host gather
    xT = np.ascontiguousarray(x.T)                 # [E, S]

    w_both = np.concatenate([np.asarray(w_ih_f, np.float32),
                             np.asarray(w_ih_b, np.float32)], axis=0)  # [GB, E]
    wT = np.ascontiguousarray(w_both.T)            # [E, GB]
    b_both = np.concatenate([np.asarray(b_f, np.float32),
                             np.asarray(b_b, np.float32)])             # [GB]

    # ---- NEFF A: input projections, time-sharded over 8 cores ----
    nc_a = _build_proj()
    in_maps = [{"wT": wT, "bias": b_both,
                "xT": np.ascontiguousarray(xT[:, c * SC:(c + 1) * SC])}
               for c in range(NCORES)]
    res_a = run_bass_kernel_spmd(nc_a, in_maps, core_ids=list(range(NCORES)))
    EXEC_NS['proj'] = res_a.exec_time_ns
    if os.environ.get('BASS_BENCH'):
        t0 = time.time()
        run_bass_kernel_spmd(nc_a, in_maps, core_ids=list(range(NCORES)))
        EXEC_NS['proj_wall2'] = int((time.time() - t0) * 1e9)
    xpT_all = np.concatenate([res_a.results[c]["xpT"] for c in range(NCORES)], axis=1)  # [GB, S]
    xpT_f = np.ascontiguousarray(xpT_all[:G])
    xpT_b = np.ascontiguousarray(xpT_all[G:, ::-1])   # reversed time for bwd

    # ---- NEFF B: the two recurrences on cores 0 (fwd) and 1 (bwd) ----
    nc_b = _build_rec()
    h0 = np.asarray(h0, np.float32)
    c0 = np.asarray(c0, np.float32)
    maps_b = []
    for (xpT_d, whh, hrow, crow) in ((xpT_f, w_hh_f, h0[0], c0[0]),
                                     (xpT_b, w_hh_b, h0[1], c0[1])):
        maps_b.append({
            "xpT": xpT_d,
            "whhT": np.ascontiguousarray(np.asarray(whh, np.float32).T),
            "hc0": np.ascontiguousarray(
                np.concatenate([_arr_hidden(hrow), _arr_hidden(crow)], axis=1)),
        })
    res_b = run_bass_kernel_spmd(nc_b, maps_b, core_ids=[0, 1])
    EXEC_NS['rec'] = res_b.exec_time_ns
    if os.environ.get('BASS_BENCH'):
        t0 = time.time()
        run_bass_kernel_spmd(nc_b, maps_b, core_ids=[0, 1])
        EXEC_NS['rec_wall2'] = int((time.time() - t0) * 1e9)
    hf = _decode_hsT(res_b.results[0]["hsT"])          # [S, Hh]
    hb = _decode_hsT(res_b.results[1]["hsT"])[::-1]    # un-reverse

    # ---- host epilogue: output projection + Viterbi + backtrace ----
    feats = np.concatenate([hf, hb], axis=1) @ np.asarray(W_out, np.float32).T \
        + np.asarray(b_out, np.float32)                # [S, T]
    trans = np.asarray(transitions, np.float32)
    fv = np.full(T, NEG, np.float32)
    fv[START] = 0.0
    bptrs = np.zeros((S, T), np.int32)
    for t in range(S):
        scores = fv[None, :] + trans                   # [next, prev]
        bptrs[t] = scores.argmax(1)
        fv = scores.max(1).astype(np.float32) + feats[t]
    terminal = fv + trans[STOP]
    best = int(terminal.argmax())
    score = np.float32(terminal[best])
    path = np.zeros(S, np.int32)
    b = best
    for t in range(S - 1, -1, -1):
        path[t] = b
        b = bptrs[t, b]
    return np.array(score, np.float32), path
